# revision 13
# baseline (speedup 1.0000x reference)
"""Trainium2 Bass kernel for BlockAttnRes.compute_all_inputs (v2).

Math: for each row (b,t), layer l attends over a small per-row source stack
(embedding, completed block sums S_k, and the running partial sum). Every
source is a prefix-sum of the 25 "raw" per-row vectors X = [emb, f_0..f_23],
i.e. sources V = M @ X for a constant 0/1 matrix M (25x25). The output
h_l = (A M) @ X, and scores/norms come from V @ qw^T and diag(V V^T).

v2 device layout (batches of R=5 rows, partition p = r*25 + j, P=125):
  1. one DMA loads X [128, 2048] bf16 (rows padded; HBM pitch-padded so the
     128 per-row descriptors spread across all 16 SDMA engines)
  2. PE pass 1 folds M during the transpose: VT_c = X_c.T @ mtbd per d-chunk
     (16 matmuls, N=128) -> PSUM; ACT/DVE copy VT to SBUF bf16
  3. PE pass 2: SG += VT_c.T @ [VT_c | qwT_c] (16 matmuls, N=152) giving the
     V-Gram (diag = ||v_n||^2) and scores v_n.qw_l in one accumulation
  4. DVE masked-diag reduce -> z; ACT rsqrt via exp(-0.5*ln(z/D+eps));
     masked softmax over sources (tiny transposed ops, as v1)
  5. alphas folded through M on PE (BT = mbd.T @ abd); H = BT.T @ X bf16
  6. H PSUM -> SBUF bf16 -> one DMA to pitch-padded outT (16-engine spread)

All HBM I/O is bf16 (halves traffic vs v1); host up/down-converts.
Sharding: data-parallel over B*T = 2048 rows -> 8 cores x 256 rows.
"""

import numpy as np
import ml_dtypes

import concourse.bass as bass
import concourse.bacc as bacc
import concourse.mybir as mybir
from concourse import tile
from concourse.alu_op_type import AluOpType
from concourse.bass_utils import run_bass_kernel_spmd

L = 24
D = 2048
NUM_BLOCKS = 8
EPS = 1e-6
B, T = 2, 1024
N_CORES = 8

ROWS_PER_CORE = (B * T) // N_CORES  # 256
R = 5            # rows per batch
NJ = 25          # raw vectors per row: emb + 24 layer outputs
NS = 25          # sources per row
P = NJ * R       # 125 live partitions per batch
NCHUNK = D // 128  # 16 d-chunks
CW = 152         # vt_sb column stride per chunk: 128 VT cols + 24 qw
XF = D + 32      # padded HBM row pitch (prevents descriptor flat-merge)
NEG = -1e30

f32 = mybir.dt.float32
bf16 = mybir.dt.bfloat16


def _source_matrix():
    """M[n, j]: source n = sum_j M[n,j] * raw_j. Raw j=0 is emb, j=1+l is f_l.
    Sources: n=0 emb; n=1+3k+i (i=0,1,2) is C_{k,i+1} = f_{3k}+..+f_{3k+i}."""
    M = np.zeros((NS, NJ), dtype=np.float32)
    M[0, 0] = 1.0
    for k in range(NUM_BLOCKS):
        for i in range(3):
            n = 1 + 3 * k + i
            M[n, 1 + 3 * k : 1 + 3 * k + i + 1] = 1.0
    return M


def _valid_matrix():
    """valid[l, n]: which sources layer l attends over (block k=l//3, i=l%3)."""
    V = np.zeros((L, NS), dtype=bool)
    for l in range(L):
        kb, ii = l // 3, l % 3
        V[l, 0] = True
        for k in range(kb):
            V[l, 3 * k + 3] = True
        if ii > 0:
            V[l, 3 * kb + ii] = True
    return V


def _build_consts(queries, key_norm_weight):
    M = _source_matrix()
    valid = _valid_matrix()
    eye_r = np.eye(R, dtype=np.float32)

    qw = (queries * key_norm_weight[None, :]).astype(np.float32)  # [L, D]
    # qwT[p, c*24 + l] = qw[l, c*128 + p]
    qwT = np.ascontiguousarray(
        qw.reshape(L, NCHUNK, 128).transpose(2, 1, 0).reshape(128, NCHUNK * L)
    ).astype(ml_dtypes.bfloat16)

    # mtbd[(r,j),(r',n)] = (r==r') * M[n,j]; cols padded 125->128 (zeros).
    # lhsT = X_c with rhs = mtbd gives VT_c = (M X)^T chunk directly.
    mtbd = np.einsum("nj,ab->ajbn", M, eye_r).reshape(P, NS * R)
    mtbdP = np.zeros((P, 128), dtype=np.float32)
    mtbdP[:, :P] = mtbd
    mtbdP = np.ascontiguousarray(mtbdP).astype(ml_dtypes.bfloat16)

    # mbd[(r,n),(r',j)] = (r==r') * M[n,j]; cols padded 125->128 (zeros).
    # BT = mbd.T @ abd folds alphas back to raw-vector space.
    mbd = np.einsum("nj,ab->anbj", M, eye_r).reshape(NS * R, P)
    mbdP = np.zeros((P, 128), dtype=np.float32)
    mbdP[:, :P] = mbd
    mbdP = np.ascontiguousarray(mbdP).astype(ml_dtypes.bfloat16)

    # eyemask[(r,n), col] = 1 iff col == (r,n): extracts diag(V V^T) via
    # masked row-sum (z = sum_col SG[:, 0:128] * eyemask)
    eyemask = np.zeros((P, 128), dtype=np.float32)
    eyemask[:, :P] = np.eye(P, dtype=np.float32)

    # diagm[(r,n),(r',l)] = (r==r')
    diagm = np.einsum("ab,nl->anbl", eye_r, np.ones((NS, L), np.float32))
    diagm = np.ascontiguousarray(diagm.reshape(P, R * L)).astype(np.float32)
    # maskneg[l, (r,n)] = 0 if valid else NEG
    maskneg = np.where(valid[:, None, :], 0.0, NEG)
    maskneg = np.broadcast_to(maskneg, (L, R, NS)).reshape(L, R * NS)
    maskneg = np.ascontiguousarray(maskneg).astype(np.float32)

    ident = np.eye(128, dtype=np.float32)
    return dict(qwT=qwT, mtbdP=mtbdP, mbdP=mbdP, eyemask=eyemask,
                diagm=diagm, maskneg=maskneg, ident=ident)


def _batch_starts():
    starts = [R * b for b in range(ROWS_PER_CORE // R)]  # 0..250
    if starts[-1] + R < ROWS_PER_CORE:
        starts.append(ROWS_PER_CORE - R)  # 251 (overlaps; identical rewrites)
    return starts


def build_kernel():
    nc = bacc.Bacc("TRN2", target_bir_lowering=False, debug=False)

    # host-transposed input: [row, j, d] rows at pitch XF, bf16, +3 tail rows
    # so each batch can load a full 128 partitions (tail values are dead:
    # btsb rows 125:128 are zero).
    loT = nc.dram_tensor("loT", [ROWS_PER_CORE * NJ + 3, XF], bf16,
                         kind="ExternalInput").ap()
    qwT_d = nc.dram_tensor("qwT", [128, NCHUNK * L], bf16, kind="ExternalInput").ap()
    mtbd_d = nc.dram_tensor("mtbdP", [P, 128], bf16, kind="ExternalInput").ap()
    mbd_d = nc.dram_tensor("mbdP", [P, 128], bf16, kind="ExternalInput").ap()
    eyem_d = nc.dram_tensor("eyemask", [P, 128], f32, kind="ExternalInput").ap()
    diagm_d = nc.dram_tensor("diagm", [P, R * L], f32, kind="ExternalInput").ap()
    maskneg_d = nc.dram_tensor("maskneg", [L, R * NS], f32, kind="ExternalInput").ap()
    ident_d = nc.dram_tensor("ident", [128, 128], f32, kind="ExternalInput").ap()
    # output [row, l, d] bf16 at pitch XF (pad cols never written; host strips).
    # The pitch gap keeps the 120 write descriptors unmerged -> 16 engines.
    outT = nc.dram_tensor("outT", [ROWS_PER_CORE * L, XF], bf16,
                          kind="ExternalOutput").ap()

    with tile.TileContext(nc) as tc:
        with (
            tc.tile_pool(name="const", bufs=1) as const,
            tc.tile_pool(name="xpool", bufs=4) as xpool,

            tc.tile_pool(name="hpool", bufs=3) as hpool,
            tc.tile_pool(name="small", bufs=2) as small,
            tc.tile_pool(name="ps_xt", bufs=3, space=bass.MemorySpace.PSUM) as ps_xt,
            tc.tile_pool(name="ps_sg", bufs=2, space=bass.MemorySpace.PSUM) as ps_sg,
            tc.tile_pool(name="ps_sm", bufs=1, space=bass.MemorySpace.PSUM) as ps_sm,
            tc.tile_pool(name="ps_h", bufs=2, space=bass.MemorySpace.PSUM) as ps_h,
        ):
            qwT = const.tile([128, NCHUNK * L], bf16)
            nc.sync.dma_start(qwT[:], qwT_d[:])
            mtbd = const.tile([P, 128], bf16)
            nc.sync.dma_start(mtbd[:], mtbd_d[:])
            mbd = const.tile([P, 128], bf16)
            nc.sync.dma_start(mbd[:], mbd_d[:])
            eyem = const.tile([P, 128], f32)
            nc.sync.dma_start(eyem[:], eyem_d[:])
            diagm = const.tile([P, R * L], f32)
            nc.sync.dma_start(diagm[:], diagm_d[:])
            maskneg = const.tile([L, R * NS], f32)
            nc.sync.dma_start(maskneg[:], maskneg_d[:])
            ident = const.tile([128, 128], f32)
            nc.sync.dma_start(ident[:], ident_d[:])
            epsb = const.tile([P, 1], f32)
            nc.vector.memset(epsb[:], EPS)

            # Three fixed vt buffers, rotated manually across batches. The
            # constant qw columns are written once into each; per-batch work
            # only rewrites the VT columns (dep tracking serializes reuse).
            vtbufs = []
            for i in range(3):
                vtb = const.tile([128, NCHUNK * CW], bf16, name=f"vtbuf{i}")
                nc.vector.tensor_copy(
                    vtb.rearrange("p (c w) -> p c w", w=CW)[:, :, 128:CW],
                    qwT.rearrange("p (c w) -> p c w", w=L),
                )
                vtbufs.append(vtb)

            # ---------------- software-pipelined emission ----------------
            # PE executes its queue strictly in order, so batch i's tail ops
            # (scoreT/alphaT/BT/H) are interleaved into batch i+1's
            # pass1/pass2 stream; their cross-engine deps resolve while the
            # big matmuls run. DMA prefetch runs two batches ahead.
            starts = _batch_starts()
            nb = len(starts)
            st = {}  # per-batch live tiles

            def emit_dma_in(i):
                X = xpool.tile([128, D], bf16, name=f"X{i}", tag="X", bufs=5)
                nc.sync.dma_start(
                    X[:], loT[starts[i] * NJ : starts[i] * NJ + 128, 0:D]
                )
                st[i] = {"X": X}

            def emit_pass1(i):
                X = st[i]["X"]
                vt = vtbufs[i % 3]
                vt3 = vt.rearrange("p (c w) -> p c w", w=CW)
                for half in range(4):
                    xtp = ps_xt.tile([128, 512], f32, name=f"xtp{i}_{half}", tag="xtp")
                    for cc in range(4):
                        c = 4 * half + cc
                        nc.tensor.matmul(
                            xtp[:, 128 * cc : 128 * (cc + 1)],
                            X[0:P, 128 * c : 128 * (c + 1)],
                            mtbd[:],
                            start=True,
                            stop=True,
                        )
                    if half % 2 == 0:
                        nc.scalar.copy(
                            vt3[:, 4 * half : 4 * half + 4, 0:128],
                            xtp.rearrange("p (cc w) -> p cc w", w=128),
                        )
                    else:
                        nc.vector.tensor_copy(
                            vt3[:, 4 * half : 4 * half + 4, 0:128],
                            xtp.rearrange("p (cc w) -> p cc w", w=128),
                        )
                st[i]["vt"] = vt

            def emit_pass2(i):
                vt = st[i]["vt"]
                SG = ps_sg.tile([128, CW], f32, name=f"SG{i}", tag="SG")
                for c in range(NCHUNK):
                    base = CW * c
                    nc.tensor.matmul(
                        SG[:],
                        vt[:, base : base + 128],
                        vt[:, base : base + CW],
                        start=(c == 0),
                        stop=(c == NCHUNK - 1),
                    )
                st[i]["SG"] = SG

            def emit_zchain(i):
                SG = st[i]["SG"]
                junk = small.tile([P, 128], f32, name=f"junk{i}", tag="junk")
                z = small.tile([P, 1], f32, name=f"z{i}", tag="z")
                nc.vector.scalar_tensor_tensor(
                    out=junk[:],
                    in0=SG[0:P, 0:128],
                    scalar=1.0,
                    in1=eyem[:],
                    op0=AluOpType.mult,
                    op1=AluOpType.mult,
                    accum_out=z[:],
                )
                lnu = small.tile([P, 1], f32, name=f"lnu{i}", tag="lnu")
                nc.scalar.activation(
                    lnu[:], z[:], mybir.ActivationFunctionType.Ln,
                    bias=epsb[:], scale=1.0 / D,
                )
                rsq = small.tile([P, 1], f32, name=f"rsq{i}", tag="rsq")
                nc.scalar.activation(
                    rsq[:], lnu[:], mybir.ActivationFunctionType.Exp, scale=-0.5
                )
                scoresR = small.tile([P, L], f32, name=f"scoresR{i}", tag="scoresR")
                nc.scalar.activation(
                    scoresR[:], SG[0:P, 128:CW],
                    mybir.ActivationFunctionType.Copy, scale=rsq[:],
                )
                st[i]["scoresR"] = scoresR

            def emit_scoreT(i):
                scoreT = ps_sm.tile([L, P], f32, name=f"scoreT{i}", tag="sm")
                nc.tensor.transpose(scoreT[:], st[i]["scoresR"][:], ident[:P, :P])
                st[i]["scoreT"] = scoreT

            def emit_softmax(i):
                smask = small.tile([L, P], f32, name=f"smask{i}", tag="smask")
                nc.vector.tensor_add(smask[:], st[i]["scoreT"][:], maskneg[:])
                esc = small.tile([L, P], f32, name=f"esc{i}", tag="esc")
                nc.scalar.activation(
                    esc[:], smask[:], mybir.ActivationFunctionType.Exp
                )
                ssum = small.tile([L, R], f32, name=f"ssum{i}", tag="ssum")
                nc.vector.reduce_sum(
                    ssum[:],
                    esc.rearrange("p (r n) -> p r n", r=R),
                    axis=mybir.AxisListType.X,
                )
                rec = small.tile([L, R], f32, name=f"rec{i}", tag="rec")
                nc.vector.reciprocal(rec[:], ssum[:])
                alpha = small.tile([L, P], f32, name=f"alpha{i}", tag="alpha")
                nc.vector.tensor_tensor(
                    alpha.rearrange("p (r n) -> p r n", r=R),
                    esc.rearrange("p (r n) -> p r n", r=R),
                    rec.unsqueeze(2).broadcast_to([L, R, NS]),
                    AluOpType.mult,
                )
                st[i]["alpha"] = alpha

            def emit_alphaT(i):
                alphaT = ps_sm.tile([P, L], f32, name=f"alphaT{i}", tag="sm")
                nc.tensor.transpose(alphaT[:], st[i]["alpha"][:], ident[:L, :L])
                st[i]["alphaT"] = alphaT

            def emit_abd(i):
                abd = small.tile([P, R * L], bf16, name=f"abd{i}", tag="abd")
                nc.vector.scalar_tensor_tensor(
                    out=abd.rearrange("p (r l) -> p r l", r=R),
                    in0=st[i]["alphaT"].unsqueeze(1).broadcast_to([P, R, L]),
                    scalar=1.0,
                    in1=diagm.rearrange("p (r l) -> p r l", r=R),
                    op0=AluOpType.mult,
                    op1=AluOpType.mult,
                )
                st[i]["abd"] = abd

            def emit_BT(i):
                BT = ps_sm.tile([128, R * L], f32, name=f"BT{i}", tag="sm")
                nc.tensor.matmul(BT[:], mbd[:], st[i]["abd"][:],
                                 start=True, stop=True)
                btsb = small.tile([128, R * L], bf16, name=f"btsb{i}", tag="btsb")
                nc.scalar.copy(btsb[:], BT[:])
                st[i]["btsb"] = btsb

            def emit_H(i):
                X, btsb = st[i]["X"], st[i]["btsb"]
                H_sb = hpool.tile([R * L, D], bf16, name=f"H{i}", tag="H")
                for hb in range(4):
                    Hp = ps_h.tile([R * L, 512], f32, name=f"Hp{i}_{hb}", tag="Hp")
                    nc.tensor.matmul(
                        Hp[:],
                        btsb[:],
                        X[:, 512 * hb : 512 * (hb + 1)],
                        start=True,
                        stop=True,
                    )
                    if hb % 2 == 0:
                        nc.scalar.copy(H_sb[:, 512 * hb : 512 * (hb + 1)], Hp[:])
                    else:
                        nc.vector.tensor_copy(
                            H_sb[:, 512 * hb : 512 * (hb + 1)], Hp[:]
                        )
                # out-DMA on the ACT HWDGE ring; pitch-padded rows spread
                # across 16 engines
                nc.scalar.dma_start(
                    outT[starts[i] * L : starts[i] * L + R * L, 0:D], H_sb[:]
                )
                del st[i]

            emit_dma_in(0)
            if nb > 1:
                emit_dma_in(1)
            for it in range(nb):
                if it >= 1:
                    emit_scoreT(it - 1)
                    emit_softmax(it - 1)
                if it + 2 < nb:
                    emit_dma_in(it + 2)
                emit_pass1(it)
                if it >= 1:
                    emit_alphaT(it - 1)
                    emit_abd(it - 1)
                emit_pass2(it)
                if it >= 1:
                    emit_BT(it - 1)
                emit_zchain(it)
                if it >= 1:
                    emit_H(it - 1)
            last = nb - 1
            emit_scoreT(last)
            emit_softmax(last)
            emit_alphaT(last)
            emit_abd(last)
            emit_BT(last)
            emit_H(last)

    # Pin Ln/Exp to the one table set containing both, so the compiled stream
    # has a single ACT table load instead of two reloads (~2.7us) per batch.
    real_gat = bacc.get_activation_tables
    AF = mybir.ActivationFunctionType

    def gat_pinned(arch):
        out = {}
        for name, fns in real_gat(arch).items():
            if name == "natural_log_exp_and_others":
                out[name] = set(fns)
            else:
                out[name] = {f for f in fns if f not in (AF.Ln, AF.Exp)}
        return out

    bacc.get_activation_tables = gat_pinned
    try:
        nc.compile()
    finally:
        bacc.get_activation_tables = real_gat
    return nc


_NC_CACHE = None


def _prep_loT(layer_outputs, embedding):
    """[L,B,T,D]+[B,T,D] -> per-row stacks [B*T*25 (+3), XF] bf16 (row-major,
    rows padded to the XF pitch, 3 zero tail rows for the last batch)."""
    lo_flat = layer_outputs.reshape(L, B * T, D)
    emb_flat = embedding.reshape(B * T, D)
    loT = np.zeros((B * T * NJ + 3, XF), dtype=ml_dtypes.bfloat16)
    v = loT[: B * T * NJ].reshape(B * T, NJ, XF)
    v[:, 0, :D] = emb_flat.astype(ml_dtypes.bfloat16)
    v[:, 1:, :D] = lo_flat.transpose(1, 0, 2).astype(ml_dtypes.bfloat16)
    return loT


def _make_in_maps(loT, consts):
    in_maps = []
    for c in range(N_CORES):
        r0 = c * ROWS_PER_CORE * NJ
        in_maps.append({
            "loT": loT[r0 : r0 + ROWS_PER_CORE * NJ + 3],
            "qwT": consts["qwT"],
            "mtbdP": consts["mtbdP"],
            "mbdP": consts["mbdP"],
            "eyemask": consts["eyemask"],
            "diagm": consts["diagm"],
            "maskneg": consts["maskneg"],
            "ident": consts["ident"],
        })
    return in_maps


def kernel(layer_outputs, embedding, queries, key_norm_weight):
    global _NC_CACHE
    layer_outputs = np.asarray(layer_outputs, dtype=np.float32)
    embedding = np.asarray(embedding, dtype=np.float32)
    queries = np.asarray(queries, dtype=np.float32)
    key_norm_weight = np.asarray(key_norm_weight, dtype=np.float32)

    loT = _prep_loT(layer_outputs, embedding)
    consts = _build_consts(queries, key_norm_weight)

    if _NC_CACHE is None:
        _NC_CACHE = build_kernel()
    nc = _NC_CACHE

    in_maps = _make_in_maps(loT, consts)
    res = run_bass_kernel_spmd(nc, in_maps, core_ids=list(range(N_CORES)))

    full = np.empty((L, B * T, D), dtype=np.float32)
    for c in range(N_CORES):
        r0 = c * ROWS_PER_CORE
        outT = res.results[c]["outT"][:, :D].astype(np.float32)
        outT = outT.reshape(ROWS_PER_CORE, L, D)
        full[:, r0 : r0 + ROWS_PER_CORE, :] = outT.transpose(1, 0, 2)
    return full.reshape(L, B, T, D)


# revision 14
# speedup vs baseline: 1.1572x; 1.1572x over previous
"""Trainium2 Bass kernel for BlockAttnRes.compute_all_inputs (v2).

Math: for each row (b,t), layer l attends over a small per-row source stack
(embedding, completed block sums S_k, and the running partial sum). Every
source is a prefix-sum of the 25 "raw" per-row vectors X = [emb, f_0..f_23],
i.e. sources V = M @ X for a constant 0/1 matrix M (25x25). The output
h_l = (A M) @ X, and scores/norms come from V @ qw^T and diag(V V^T).

v2 device layout (batches of R=5 rows, partition p = r*25 + j, P=125):
  1. one DMA loads X [128, 2048] bf16 (rows padded; HBM pitch-padded so the
     128 per-row descriptors spread across all 16 SDMA engines)
  2. PE pass 1 folds M during the transpose: VT_c = X_c.T @ mtbd per d-chunk
     (16 matmuls, N=128) -> PSUM; ACT/DVE copy VT to SBUF bf16
  3. PE pass 2: SG += VT_c.T @ [VT_c | qwT_c] (16 matmuls, N=152) giving the
     V-Gram (diag = ||v_n||^2) and scores v_n.qw_l in one accumulation
  4. DVE masked-diag reduce -> z; ACT rsqrt via exp(-0.5*ln(z/D+eps));
     masked softmax over sources (tiny transposed ops, as v1)
  5. alphas folded through M on PE (BT = mbd.T @ abd); H = BT.T @ X bf16
  6. H PSUM -> SBUF bf16 -> one DMA to pitch-padded outT (16-engine spread)

All HBM I/O is bf16 (halves traffic vs v1); host up/down-converts.
Sharding: data-parallel over B*T = 2048 rows -> 8 cores x 256 rows.
"""

import numpy as np
import ml_dtypes

import concourse.bass as bass
import concourse.bacc as bacc
import concourse.mybir as mybir
from concourse import tile
from concourse.alu_op_type import AluOpType
from concourse.bass_utils import run_bass_kernel_spmd

L = 24
D = 2048
NUM_BLOCKS = 8
EPS = 1e-6
B, T = 2, 1024
N_CORES = 8

ROWS_PER_CORE = (B * T) // N_CORES  # 256
R = 5            # rows per batch
NJ = 25          # raw vectors per row: emb + 24 layer outputs
NS = 25          # sources per row
P = NJ * R       # 125 live partitions per batch
NCHUNK = D // 128  # 16 d-chunks
CW = 152         # vt_sb column stride per chunk: 128 VT cols + 24 qw
XF = D + 32      # padded HBM row pitch (prevents descriptor flat-merge)
NEG = -1e30

f32 = mybir.dt.float32
bf16 = mybir.dt.bfloat16


def _source_matrix():
    """M[n, j]: source n = sum_j M[n,j] * raw_j. Raw j=0 is emb, j=1+l is f_l.
    Sources: n=0 emb; n=1+3k+i (i=0,1,2) is C_{k,i+1} = f_{3k}+..+f_{3k+i}."""
    M = np.zeros((NS, NJ), dtype=np.float32)
    M[0, 0] = 1.0
    for k in range(NUM_BLOCKS):
        for i in range(3):
            n = 1 + 3 * k + i
            M[n, 1 + 3 * k : 1 + 3 * k + i + 1] = 1.0
    return M


def _valid_matrix():
    """valid[l, n]: which sources layer l attends over (block k=l//3, i=l%3)."""
    V = np.zeros((L, NS), dtype=bool)
    for l in range(L):
        kb, ii = l // 3, l % 3
        V[l, 0] = True
        for k in range(kb):
            V[l, 3 * k + 3] = True
        if ii > 0:
            V[l, 3 * kb + ii] = True
    return V


def _build_consts(queries, key_norm_weight):
    M = _source_matrix()
    valid = _valid_matrix()
    eye_r = np.eye(R, dtype=np.float32)

    qw = (queries * key_norm_weight[None, :]).astype(np.float32)  # [L, D]
    # qwT[p, c*24 + l] = qw[l, c*128 + p]
    qwT = np.ascontiguousarray(
        qw.reshape(L, NCHUNK, 128).transpose(2, 1, 0).reshape(128, NCHUNK * L)
    ).astype(ml_dtypes.bfloat16)

    # mtbd[(r,j),(r',n)] = (r==r') * M[n,j]; cols padded 125->128 (zeros).
    # lhsT = X_c with rhs = mtbd gives VT_c = (M X)^T chunk directly.
    mtbd = np.einsum("nj,ab->ajbn", M, eye_r).reshape(P, NS * R)
    mtbdP = np.zeros((P, 128), dtype=np.float32)
    mtbdP[:, :P] = mtbd
    mtbdP = np.ascontiguousarray(mtbdP).astype(ml_dtypes.bfloat16)

    # mbd[(r,n),(r',j)] = (r==r') * M[n,j]; cols padded 125->128 (zeros).
    # BT = mbd.T @ abd folds alphas back to raw-vector space.
    mbd = np.einsum("nj,ab->anbj", M, eye_r).reshape(NS * R, P)
    mbdP = np.zeros((P, 128), dtype=np.float32)
    mbdP[:, :P] = mbd
    mbdP = np.ascontiguousarray(mbdP).astype(ml_dtypes.bfloat16)

    # eyemask[(r,n), col] = 1 iff col == (r,n): extracts diag(V V^T) via
    # masked row-sum (z = sum_col SG[:, 0:128] * eyemask)
    eyemask = np.zeros((P, 128), dtype=np.float32)
    eyemask[:, :P] = np.eye(P, dtype=np.float32)

    # diagm[(r,n),(r',l)] = (r==r')
    diagm = np.einsum("ab,nl->anbl", eye_r, np.ones((NS, L), np.float32))
    diagm = np.ascontiguousarray(diagm.reshape(P, R * L)).astype(np.float32)
    # maskneg[l, (r,n)] = 0 if valid else NEG
    maskneg = np.where(valid[:, None, :], 0.0, NEG)
    maskneg = np.broadcast_to(maskneg, (L, R, NS)).reshape(L, R * NS)
    maskneg = np.ascontiguousarray(maskneg).astype(np.float32)

    ident = np.eye(128, dtype=np.float32)
    return dict(qwT=qwT, mtbdP=mtbdP, mbdP=mbdP, eyemask=eyemask,
                diagm=diagm, maskneg=maskneg, ident=ident)


def _batch_starts():
    starts = [R * b for b in range(ROWS_PER_CORE // R)]  # 0..250
    if starts[-1] + R < ROWS_PER_CORE:
        starts.append(ROWS_PER_CORE - R)  # 251 (overlaps; identical rewrites)
    return starts


def build_kernel():
    nc = bacc.Bacc("TRN2", target_bir_lowering=False, debug=False)

    # host-transposed input: [row, j, d] rows at pitch XF, bf16, +3 tail rows
    # so each batch can load a full 128 partitions (tail values are dead:
    # btsb rows 125:128 are zero).
    loT = nc.dram_tensor("loT", [ROWS_PER_CORE * NJ + 3, XF], bf16,
                         kind="ExternalInput").ap()
    qwT_d = nc.dram_tensor("qwT", [128, NCHUNK * L], bf16, kind="ExternalInput").ap()
    mtbd_d = nc.dram_tensor("mtbdP", [P, 128], bf16, kind="ExternalInput").ap()
    mbd_d = nc.dram_tensor("mbdP", [P, 128], bf16, kind="ExternalInput").ap()
    eyem_d = nc.dram_tensor("eyemask", [P, 128], f32, kind="ExternalInput").ap()
    diagm_d = nc.dram_tensor("diagm", [P, R * L], f32, kind="ExternalInput").ap()
    maskneg_d = nc.dram_tensor("maskneg", [L, R * NS], f32, kind="ExternalInput").ap()
    ident_d = nc.dram_tensor("ident", [128, 128], f32, kind="ExternalInput").ap()
    # output [row, l, d] bf16 at pitch XF (pad cols never written; host strips).
    # The pitch gap keeps the 120 write descriptors unmerged -> 16 engines.
    outT = nc.dram_tensor("outT", [ROWS_PER_CORE * L, XF], bf16,
                          kind="ExternalOutput").ap()

    with tile.TileContext(nc) as tc:
        with (
            tc.tile_pool(name="const", bufs=1) as const,
            tc.tile_pool(name="xpool", bufs=4) as xpool,

            tc.tile_pool(name="hpool", bufs=3) as hpool,
            tc.tile_pool(name="small", bufs=2) as small,
            tc.tile_pool(name="ps_xt", bufs=4, space=bass.MemorySpace.PSUM) as ps_xt,
            tc.tile_pool(name="ps_sg", bufs=1, space=bass.MemorySpace.PSUM) as ps_sg,
            tc.tile_pool(name="ps_sm", bufs=1, space=bass.MemorySpace.PSUM) as ps_sm,
            tc.tile_pool(name="ps_h", bufs=2, space=bass.MemorySpace.PSUM) as ps_h,
        ):
            qwT = const.tile([128, NCHUNK * L], bf16)
            nc.sync.dma_start(qwT[:], qwT_d[:])
            mtbd = const.tile([P, 128], bf16)
            nc.sync.dma_start(mtbd[:], mtbd_d[:])
            mbd = const.tile([P, 128], bf16)
            nc.sync.dma_start(mbd[:], mbd_d[:])
            eyem = const.tile([P, 128], f32)
            nc.sync.dma_start(eyem[:], eyem_d[:])
            diagm = const.tile([P, R * L], f32)
            nc.sync.dma_start(diagm[:], diagm_d[:])
            maskneg = const.tile([L, R * NS], f32)
            nc.sync.dma_start(maskneg[:], maskneg_d[:])
            ident = const.tile([128, 128], f32)
            nc.sync.dma_start(ident[:], ident_d[:])
            epsb = const.tile([P, 1], f32)
            nc.vector.memset(epsb[:], EPS)

            # Three fixed vt buffers, rotated manually across batches. The
            # constant qw columns are written once into each; per-batch work
            # only rewrites the VT columns (dep tracking serializes reuse).
            vtbufs = []
            for i in range(3):
                vtb = const.tile([128, NCHUNK * CW], bf16, name=f"vtbuf{i}")
                nc.vector.tensor_copy(
                    vtb.rearrange("p (c w) -> p c w", w=CW)[:, :, 128:CW],
                    qwT.rearrange("p (c w) -> p c w", w=L),
                )
                vtbufs.append(vtb)

            # ---------------- software-pipelined emission ----------------
            # PE executes its queue strictly in order, so batch i's tail ops
            # (scoreT/alphaT/BT/H) are interleaved into batch i+1's
            # pass1/pass2 stream; their cross-engine deps resolve while the
            # big matmuls run. DMA prefetch runs two batches ahead.
            starts = _batch_starts()
            nb = len(starts)
            st = {}  # per-batch live tiles

            def emit_dma_in(i):
                X = xpool.tile([128, D], bf16, name=f"X{i}", tag="X", bufs=5)
                nc.sync.dma_start(
                    X[:], loT[starts[i] * NJ : starts[i] * NJ + 128, 0:D]
                )
                st[i] = {"X": X}

            def emit_pass1(i):
                X = st[i]["X"]
                vt = vtbufs[i % 3]
                vt3 = vt.rearrange("p (c w) -> p c w", w=CW)
                for half in range(4):
                    xtp = ps_xt.tile([128, 512], f32, name=f"xtp{i}_{half}", tag="xtp")
                    for cc in range(4):
                        c = 4 * half + cc
                        nc.tensor.matmul(
                            xtp[:, 128 * cc : 128 * (cc + 1)],
                            X[0:P, 128 * c : 128 * (c + 1)],
                            mtbd[:],
                            start=True,
                            stop=True,
                        )
                    if half % 2 == 0:
                        nc.scalar.copy(
                            vt3[:, 4 * half : 4 * half + 4, 0:128],
                            xtp.rearrange("p (cc w) -> p cc w", w=128),
                        )
                    else:
                        nc.vector.tensor_copy(
                            vt3[:, 4 * half : 4 * half + 4, 0:128],
                            xtp.rearrange("p (cc w) -> p cc w", w=128),
                        )
                st[i]["vt"] = vt

            def emit_pass2(i):
                vt = st[i]["vt"]
                SG = ps_sg.tile([128, CW], f32, name=f"SG{i}", tag="SG")
                for c in range(NCHUNK):
                    base = CW * c
                    nc.tensor.matmul(
                        SG[:],
                        vt[:, base : base + 128],
                        vt[:, base : base + CW],
                        start=(c == 0),
                        stop=(c == NCHUNK - 1),
                    )
                st[i]["SG"] = SG

            def emit_zchain(i):
                SG = st[i]["SG"]
                junk = small.tile([P, 128], f32, name=f"junk{i}", tag="junk")
                z = small.tile([P, 1], f32, name=f"z{i}", tag="z")
                nc.vector.scalar_tensor_tensor(
                    out=junk[:],
                    in0=SG[0:P, 0:128],
                    scalar=1.0,
                    in1=eyem[:],
                    op0=AluOpType.mult,
                    op1=AluOpType.mult,
                    accum_out=z[:],
                )
                lnu = small.tile([P, 1], f32, name=f"lnu{i}", tag="lnu")
                nc.scalar.activation(
                    lnu[:], z[:], mybir.ActivationFunctionType.Ln,
                    bias=epsb[:], scale=1.0 / D,
                )
                rsq = small.tile([P, 1], f32, name=f"rsq{i}", tag="rsq")
                nc.scalar.activation(
                    rsq[:], lnu[:], mybir.ActivationFunctionType.Exp, scale=-0.5
                )
                scoresR = small.tile([P, L], f32, name=f"scoresR{i}", tag="scoresR")
                nc.scalar.activation(
                    scoresR[:], SG[0:P, 128:CW],
                    mybir.ActivationFunctionType.Copy, scale=rsq[:],
                )
                st[i]["scoresR"] = scoresR

            def emit_scoreT(i):
                scoreT = ps_sm.tile([L, P], f32, name=f"scoreT{i}", tag="sm")
                nc.tensor.transpose(scoreT[:], st[i]["scoresR"][:], ident[:P, :P])
                st[i]["scoreT"] = scoreT

            def emit_softmax(i):
                smask = small.tile([L, P], f32, name=f"smask{i}", tag="smask")
                nc.vector.tensor_add(smask[:], st[i]["scoreT"][:], maskneg[:])
                esc = small.tile([L, P], f32, name=f"esc{i}", tag="esc")
                nc.scalar.activation(
                    esc[:], smask[:], mybir.ActivationFunctionType.Exp
                )
                ssum = small.tile([L, R], f32, name=f"ssum{i}", tag="ssum")
                nc.vector.reduce_sum(
                    ssum[:],
                    esc.rearrange("p (r n) -> p r n", r=R),
                    axis=mybir.AxisListType.X,
                )
                rec = small.tile([L, R], f32, name=f"rec{i}", tag="rec")
                nc.vector.reciprocal(rec[:], ssum[:])
                alpha = small.tile([L, P], f32, name=f"alpha{i}", tag="alpha")
                nc.vector.tensor_tensor(
                    alpha.rearrange("p (r n) -> p r n", r=R),
                    esc.rearrange("p (r n) -> p r n", r=R),
                    rec.unsqueeze(2).broadcast_to([L, R, NS]),
                    AluOpType.mult,
                )
                st[i]["alpha"] = alpha

            def emit_alphaT(i):
                alphaT = ps_sm.tile([P, L], f32, name=f"alphaT{i}", tag="sm")
                nc.tensor.transpose(alphaT[:], st[i]["alpha"][:], ident[:L, :L])
                st[i]["alphaT"] = alphaT

            def emit_abd(i):
                abd = small.tile([P, R * L], bf16, name=f"abd{i}", tag="abd")
                nc.vector.scalar_tensor_tensor(
                    out=abd.rearrange("p (r l) -> p r l", r=R),
                    in0=st[i]["alphaT"].unsqueeze(1).broadcast_to([P, R, L]),
                    scalar=1.0,
                    in1=diagm.rearrange("p (r l) -> p r l", r=R),
                    op0=AluOpType.mult,
                    op1=AluOpType.mult,
                )
                st[i]["abd"] = abd

            def emit_BT(i):
                BT = ps_sm.tile([128, R * L], f32, name=f"BT{i}", tag="sm")
                nc.tensor.matmul(BT[:], mbd[:], st[i]["abd"][:],
                                 start=True, stop=True)
                btsb = small.tile([128, R * L], bf16, name=f"btsb{i}", tag="btsb")
                nc.scalar.copy(btsb[:], BT[:])
                st[i]["btsb"] = btsb

            def emit_H(i):
                X, btsb = st[i]["X"], st[i]["btsb"]
                H_sb = hpool.tile([R * L, D], bf16, name=f"H{i}", tag="H")
                for hb in range(4):
                    Hp = ps_h.tile([R * L, 512], f32, name=f"Hp{i}_{hb}", tag="Hp")
                    nc.tensor.matmul(
                        Hp[:],
                        btsb[:],
                        X[:, 512 * hb : 512 * (hb + 1)],
                        start=True,
                        stop=True,
                    )
                    if hb % 2 == 0:
                        nc.scalar.copy(H_sb[:, 512 * hb : 512 * (hb + 1)], Hp[:])
                    else:
                        nc.vector.tensor_copy(
                            H_sb[:, 512 * hb : 512 * (hb + 1)], Hp[:]
                        )
                # out-DMA on the ACT HWDGE ring; pitch-padded rows spread
                # across 16 engines
                nc.scalar.dma_start(
                    outT[starts[i] * L : starts[i] * L + R * L, 0:D], H_sb[:]
                )
                del st[i]

            # lag-2 pipeline: iteration i runs pass1/pass2 for batch i, the
            # score-softmax stage for batch i-1, and the fold/H stage for
            # batch i-2 — every cross-engine dep is at least one full
            # iteration old, so engine queues never block on fresh work.
            emit_dma_in(0)
            if nb > 1:
                emit_dma_in(1)
            for it in range(nb):
                if it >= 1:
                    emit_scoreT(it - 1)
                    emit_softmax(it - 1)
                if it + 2 < nb:
                    emit_dma_in(it + 2)
                emit_pass1(it)
                if it >= 2:
                    emit_alphaT(it - 2)
                    emit_abd(it - 2)
                emit_pass2(it)
                if it >= 2:
                    emit_BT(it - 2)
                emit_zchain(it)
                if it >= 2:
                    emit_H(it - 2)
            for j in (nb - 1,):
                emit_scoreT(j)
                emit_softmax(j)
            for j in (nb - 2, nb - 1):
                emit_alphaT(j)
                emit_abd(j)
                emit_BT(j)
                emit_H(j)

    # Pin Ln/Exp to the one table set containing both, so the compiled stream
    # has a single ACT table load instead of two reloads (~2.7us) per batch.
    real_gat = bacc.get_activation_tables
    AF = mybir.ActivationFunctionType

    def gat_pinned(arch):
        out = {}
        for name, fns in real_gat(arch).items():
            if name == "natural_log_exp_and_others":
                out[name] = set(fns)
            else:
                out[name] = {f for f in fns if f not in (AF.Ln, AF.Exp)}
        return out

    bacc.get_activation_tables = gat_pinned
    try:
        nc.compile()
    finally:
        bacc.get_activation_tables = real_gat
    return nc


_NC_CACHE = None


def _prep_loT(layer_outputs, embedding):
    """[L,B,T,D]+[B,T,D] -> per-row stacks [B*T*25 (+3), XF] bf16 (row-major,
    rows padded to the XF pitch, 3 zero tail rows for the last batch)."""
    lo_flat = layer_outputs.reshape(L, B * T, D)
    emb_flat = embedding.reshape(B * T, D)
    loT = np.zeros((B * T * NJ + 3, XF), dtype=ml_dtypes.bfloat16)
    v = loT[: B * T * NJ].reshape(B * T, NJ, XF)
    v[:, 0, :D] = emb_flat.astype(ml_dtypes.bfloat16)
    v[:, 1:, :D] = lo_flat.transpose(1, 0, 2).astype(ml_dtypes.bfloat16)
    return loT


def _make_in_maps(loT, consts):
    in_maps = []
    for c in range(N_CORES):
        r0 = c * ROWS_PER_CORE * NJ
        in_maps.append({
            "loT": loT[r0 : r0 + ROWS_PER_CORE * NJ + 3],
            "qwT": consts["qwT"],
            "mtbdP": consts["mtbdP"],
            "mbdP": consts["mbdP"],
            "eyemask": consts["eyemask"],
            "diagm": consts["diagm"],
            "maskneg": consts["maskneg"],
            "ident": consts["ident"],
        })
    return in_maps


def kernel(layer_outputs, embedding, queries, key_norm_weight):
    global _NC_CACHE
    layer_outputs = np.asarray(layer_outputs, dtype=np.float32)
    embedding = np.asarray(embedding, dtype=np.float32)
    queries = np.asarray(queries, dtype=np.float32)
    key_norm_weight = np.asarray(key_norm_weight, dtype=np.float32)

    loT = _prep_loT(layer_outputs, embedding)
    consts = _build_consts(queries, key_norm_weight)

    if _NC_CACHE is None:
        _NC_CACHE = build_kernel()
    nc = _NC_CACHE

    in_maps = _make_in_maps(loT, consts)
    res = run_bass_kernel_spmd(nc, in_maps, core_ids=list(range(N_CORES)))

    full = np.empty((L, B * T, D), dtype=np.float32)
    for c in range(N_CORES):
        r0 = c * ROWS_PER_CORE
        outT = res.results[c]["outT"][:, :D].astype(np.float32)
        outT = outT.reshape(ROWS_PER_CORE, L, D)
        full[:, r0 : r0 + ROWS_PER_CORE, :] = outT.transpose(1, 0, 2)
    return full.reshape(L, B, T, D)
